# revision 1
# baseline (speedup 1.0000x reference)
"""Trainium2 Bass kernel for nn_AttentionBlock (pre-LN transformer block), v2.

Reference (per sequence of S=2048, D=2048, H=16 heads, hd=128):
    h  = LN1(x);  q,k,v = h @ W{q,k,v}.T + b
    o  = causal_softmax(q k^T / sqrt(hd)) v
    hp = h + o @ Wo.T + bo
    h2 = LN2(hp); out = h2 + gelu_tanh(h2 @ W1.T + b1) @ W2.T + b2

Sharding over 8 NeuronCores (core c owns flattened rows [512c,512c+512)
and head-pair {2c, 2c+1} of both sequences):
  * LN1/residual/LN2/MLP are row-parallel (512 rows/core, full weights)
  * each core computes q,k,v for its OWN rows for ALL 16 heads (full
    Wqkv, bf16) BEFORE any collective; three 8-way AllToAlls (k first,
    then q, then v) redistribute to head-sharding: core c gets heads
    {2c, 2c+1} for all 4096 rows. k goes first so QK softmax tiles can
    run ahead (RUNWAY) while v is still in flight.
  * attention per (row-block, head); O-projection is tensor-parallel
    over the core's 256 o-features, partial sums ReduceScatter-ed
    (bf16) back to row owners, interleaved into the attention stream
  * all matmul operands bf16 (PE full rate, half DMA); LN stats and
    softmax/psum accumulation in fp32
"""

import numpy as np
from contextlib import ExitStack

import concourse.bass as bass
import concourse.mybir as mybir
import concourse.tile as tile
from concourse.masks import make_identity

AF = mybir.ActivationFunctionType
ALU = mybir.AluOpType
f32 = mybir.dt.float32
f32r = mybir.dt.float32r
bf16 = mybir.dt.bfloat16

N_CORES = 8
B, S, D, H, HD = 2, 2048, 2048, 16, 128
NROW = B * S              # 4096 flattened rows
ROWS = NROW // N_CORES    # 512 rows per core
NB = NROW // ROWS         # 8 row-blocks (one per core)
NDT = D // 128            # 16 feature tiles of 128
DH = 4 * D                # 8192 MLP hidden
NF1 = DH // 128           # 64 hidden tiles
HL = H // N_CORES         # 2 heads per core
QT = S // ROWS            # 4 q-blocks of 512 per sequence
LN_EPS = 1e-5
INV_SQRT_HD = float(1.0 / np.sqrt(HD))
RUNWAY = 40               # attention QK softmax tiles emitted ahead of AV

REPL_GROUPS = [list(range(N_CORES))]


def _ln_stats(nc, pfx, cst, work, s1, s2, xr, t, first, last, cols=None):
    """Accumulate sum/sum-of-squares of one [128, n] tile into s1/s2."""
    sl = slice(None) if cols is None else cols
    nc.tensor.matmul(s1[:1, sl], cst["ones128r"][:], xr, start=first,
                     stop=last)
    xsq = work.tile([128, xr.shape[-1]], f32r, name=f"{pfx}_sq")
    nc.scalar.activation(out=xsq[:], in_=xr.bitcast(f32), func=AF.Square,
                         scale=1.0)
    nc.tensor.matmul(s2[:1, sl], cst["ones128r"][:], xsq[:], start=first,
                     stop=last)


def _ln_normalize(nc, tc, ctx, pfx, cst, x_get, out_b, g_row, b_row,
                  s1, s2):
    """Given stats s1/s2 (PSUM [1,512]), write normalized bf16 tiles.

    out = x * (g[p]*rstd[col]) + (b[p] - g[p]*mean[col]*rstd[col]),
    with the two [128,512] operands built by outer-product matmuls.
    """
    work = ctx.enter_context(tc.tile_pool(name=f"{pfx}nw", bufs=3))
    one = ctx.enter_context(tc.tile_pool(name=f"{pfx}o", bufs=1))
    ps_bc = ctx.enter_context(tc.tile_pool(name=f"{pfx}pb", bufs=3,
                                           space="PSUM"))
    mean = one.tile([1, 512], f32, name=f"{pfx}_mean")
    var = one.tile([1, 512], f32, name=f"{pfx}_var")
    rstd32 = one.tile([1, 512], f32, name=f"{pfx}_rstd32")
    rstd = one.tile([1, 512], f32r, name=f"{pfx}_rstd")
    msc32 = one.tile([1, 512], f32, name=f"{pfx}_msc32")
    mscr = one.tile([1, 512], f32r, name=f"{pfx}_mscr")
    nc.scalar.activation(out=mean[:], in_=s1[:], func=AF.Copy, scale=1.0 / D)
    # var = E[x^2] - mean^2
    nc.vector.tensor_mul(out=var[:], in0=mean[:], in1=mean[:])
    nc.vector.scalar_tensor_tensor(out=var[:], in0=s2[:], scalar=1.0 / D,
                                   in1=var[:], op0=ALU.mult, op1=ALU.subtract)
    nc.scalar.activation(out=rstd32[:], in_=var[:], func=AF.Sqrt,
                         bias=cst["eps"][:1, :], scale=1.0)
    nc.vector.reciprocal(out=rstd32[:], in_=rstd32[:])
    nc.scalar.activation(out=rstd[:], in_=rstd32[:], func=AF.Copy, scale=1.0)
    # mscr = -mean * rstd
    nc.vector.tensor_mul(out=msc32[:], in0=mean[:], in1=rstd32[:])
    nc.scalar.activation(out=mscr[:], in_=msc32[:],
                         func=AF.Copy, scale=-1.0)
    onesr512 = cst["ones1x512r"]
    for t in range(NDT):
        rgb = ps_bc.tile([128, 512], f32, name=f"{pfx}_rgb")
        bb = ps_bc.tile([128, 512], f32, name=f"{pfx}_bb")
        gseg = g_row[:, t * 128:(t + 1) * 128]
        bseg = b_row[:, t * 128:(t + 1) * 128]
        nc.tensor.matmul(rgb[:], gseg, rstd[:], start=True, stop=True)
        nc.tensor.matmul(bb[:], bseg, onesr512[:], start=True, stop=False)
        nc.tensor.matmul(bb[:], gseg, mscr[:], start=False, stop=True)
        tmp = work.tile([128, 512], f32, name=f"{pfx}_tmp")
        if t % 2 == 0:
            nc.vector.tensor_mul(out=tmp[:], in0=x_get(t).bitcast(f32),
                                 in1=rgb[:])
            nc.vector.tensor_add(out=out_b[:, t, :], in0=tmp[:], in1=bb[:])
        else:
            # gpsimd/Pool cannot read PSUM: stage broadcasts via Act
            rgbs = work.tile([128, 512], f32, name=f"{pfx}_rgbs")
            bbs = work.tile([128, 512], f32, name=f"{pfx}_bbs")
            nc.scalar.activation(out=rgbs[:], in_=rgb[:], func=AF.Copy,
                                 scale=1.0)
            nc.scalar.activation(out=bbs[:], in_=bb[:], func=AF.Copy,
                                 scale=1.0)
            nc.gpsimd.tensor_mul(out=tmp[:], in0=x_get(t).bitcast(f32),
                                 in1=rgbs[:])
            nc.gpsimd.tensor_add(out=out_b[:, t, :], in0=tmp[:],
                                 in1=bbs[:])


def _ln_feature_major(nc, tc, ctx, pfx, cst, x_get, out_b, g_row, b_row):
    """LayerNorm over the partition(d) axis of NDT x [128, 512] tiles."""
    work = ctx.enter_context(tc.tile_pool(name=f"{pfx}w", bufs=3))
    ps_st = ctx.enter_context(tc.tile_pool(name=f"{pfx}ps", bufs=1,
                                           space="PSUM"))
    s1 = ps_st.tile([1, 512], f32, name=f"{pfx}_s1")
    s2 = ps_st.tile([1, 512], f32, name=f"{pfx}_s2")
    for t in range(NDT):
        _ln_stats(nc, pfx, cst, work, s1, s2, x_get(t), t, t == 0,
                  t == NDT - 1)
    _ln_normalize(nc, tc, ctx, pfx, cst, x_get, out_b, g_row, b_row, s1, s2)


def _early_out(nc, tc, R, y_out):
    with ExitStack() as es:
        p = es.enter_context(tc.tile_pool(name=f"eo{R}", bufs=1))
        z = p.tile([128, 16], f32, name="eoz")
        nc.vector.memset(z[:], 0.0)
        nc.sync.dma_start(out=y_out[:128, :16], in_=z[:])


def build_nc(reps=1, upto=6, nocc=False):
    nc = bass.Bass("TRN2", target_bir_lowering=False, debug=False,
                   num_devices=N_CORES)

    # ---- kernel I/O ----
    x_in = nc.dram_tensor("x", [ROWS, D], f32, kind="ExternalInput")
    # full Wqkv, columns ordered [Wk.T | Wq.T | Wv.T], head-major features
    wqkv_in = nc.dram_tensor("wqkvT", [D, 3 * D], bf16, kind="ExternalInput")
    bqkv_in = nc.dram_tensor("bqkv", [3 * D], f32, kind="ExternalInput")
    # per-core Wo slice: Wo.T rows [256c .. +256) -> [2, 128, D]
    wo_in = nc.dram_tensor("woS", [HL, 128, D], bf16, kind="ExternalInput")
    bo_in = nc.dram_tensor("bo", [D], f32, kind="ExternalInput")
    w1_in = nc.dram_tensor("w1T", [D, DH], bf16, kind="ExternalInput")
    b1_in = nc.dram_tensor("b1", [DH], f32, kind="ExternalInput")
    w2_in = nc.dram_tensor("w2T", [DH, D], bf16, kind="ExternalInput")
    b2_in = nc.dram_tensor("b2", [D], f32, kind="ExternalInput")
    ln1g_in = nc.dram_tensor("ln1g", [D], f32, kind="ExternalInput")
    ln1b_in = nc.dram_tensor("ln1b", [D], f32, kind="ExternalInput")
    ln2g_in = nc.dram_tensor("ln2g", [D], f32, kind="ExternalInput")
    ln2b_in = nc.dram_tensor("ln2b", [D], f32, kind="ExternalInput")
    y_out = nc.dram_tensor("y", [ROWS, D], f32, kind="ExternalOutput")

    # ---- internal DRAM (collective bounce buffers) ----
    a2ak_in = nc.dram_tensor("a2ak_in", [N_CORES, HL * 128, ROWS], bf16)
    a2ak_out = nc.dram_tensor("a2ak_out", [N_CORES, HL * 128, ROWS], bf16)
    a2aq_in = nc.dram_tensor("a2aq_in", [N_CORES, HL * 128, ROWS], bf16)
    a2aq_out = nc.dram_tensor("a2aq_out", [N_CORES, HL * 128, ROWS], bf16)
    a2av_in = nc.dram_tensor("a2av_in", [N_CORES, HL * 128, ROWS], bf16)
    a2av_out = nc.dram_tensor("a2av_out", [N_CORES, HL * 128, ROWS], bf16)
    rs_in = nc.dram_tensor("rs_in", [N_CORES, D, ROWS], bf16)
    rs_out = nc.dram_tensor("rs_out", [D, ROWS], bf16)

    def cc(kind, op, ins, outs):
        if nocc:
            return
        nc.gpsimd.collective_compute(kind, op, replica_groups=REPL_GROUPS,
                                     ins=ins, outs=outs)

    with tile.TileContext(nc) as tc, ExitStack() as top:
        cpool = top.enter_context(tc.tile_pool(name="cst", bufs=1))
        cst = {}

        # constants: identities, ones, causal sub-masks
        ident32 = cpool.tile([128, 128], f32, name="ident32")
        make_identity(nc, ident32[:])
        identb = cpool.tile([128, 128], bf16, name="identb")
        nc.vector.tensor_copy(out=identb[:], in_=ident32[:])
        ones128_32 = cpool.tile([128, 1], f32, name="ones128_32")
        nc.vector.memset(ones128_32[:], 1.0)
        ones128r = cpool.tile([128, 1], f32r, name="ones128r")
        nc.vector.tensor_copy(out=ones128r[:], in_=ones128_32[:])
        ones128b = cpool.tile([128, 1], bf16, name="ones128b")
        nc.vector.tensor_copy(out=ones128b[:], in_=ones128_32[:])
        ones1x128_32 = cpool.tile([1, 128], f32, name="ones1x128_32")
        nc.vector.memset(ones1x128_32[:], 1.0)
        ones1x128r = cpool.tile([1, 128], f32r, name="ones1x128r")
        nc.vector.tensor_copy(out=ones1x128r[:], in_=ones1x128_32[:])
        cst["ones128r"] = ones128r
        cst["ones1x128r"] = ones1x128r
        ones1x512_32 = cpool.tile([1, 512], f32, name="ones1x512_32")
        nc.vector.memset(ones1x512_32[:], 1.0)
        ones1x512r = cpool.tile([1, 512], f32r, name="ones1x512r")
        nc.vector.tensor_copy(out=ones1x512r[:], in_=ones1x512_32[:])
        cst["ones1x512r"] = ones1x512r
        eps_t = cpool.tile([128, 1], f32, name="eps_t")
        nc.vector.memset(eps_t[:], LN_EPS)
        cst["eps"] = eps_t

        # LN gain/bias as f32r row tiles [1, D] (lhsT of outer products);
        # loaded per-LN-phase into a scoped pool (8 KB/partition each)
        def load_rows(pool, pfx, g_t, b_t):
            out = []
            for nm, src_t in ((f"{pfx}g", g_t), (f"{pfx}b", b_t)):
                r32 = pool.tile([1, D], f32, name=f"{nm}32")
                nc.sync.dma_start(out=r32[:], in_=src_t.ap().rearrange(
                    "(o d) -> o d", o=1))
                rr = pool.tile([1, D], f32r, name=nm)
                nc.vector.tensor_copy(out=rr[:], in_=r32[:])
                out.append(rr)
            return out

        # diagonal causal masks: mask_j[kk, qq] = 1 if kk + 128*j <= qq
        masks = []
        with ExitStack() as mtmp:
            mpool = mtmp.enter_context(tc.tile_pool(name="msk32", bufs=4))
            for j in range(4):
                m32 = mpool.tile([128, 512], f32, name=f"mask32_{j}")
                nc.gpsimd.memset(m32[:], 1.0)
                nc.gpsimd.affine_select(
                    out=m32[:], in_=m32[:], compare_op=ALU.is_ge, fill=0.0,
                    base=-128 * j, channel_multiplier=-1, pattern=[[1, 512]])
                mb = cpool.tile([128, 512], bf16, name=f"maskb_{j}")
                nc.vector.tensor_copy(out=mb[:], in_=m32[:])
                masks.append(mb)

        # per-partition bias/gain columns ([128, n/128], feature = t*128+p)
        def load_cols(name, src, n):
            t = cpool.tile([128, n // 128], f32, name=name)
            nc.sync.dma_start(out=t[:], in_=src.ap().rearrange(
                "(t p) -> p t", p=128))
            return t
        bqkv_c = load_cols("bqkv_c", bqkv_in, 3 * D)    # [128, 48] (k|q|v)
        bo_c = load_cols("bo_c", bo_in, D)              # [128, 16]
        b1_c = load_cols("b1_c", b1_in, DH)             # [128, 64]
        b2_c = load_cols("b2_c", b2_in, D)


        wqkv_src = wqkv_in.ap().rearrange("(t p) f -> p t f", p=128)
        w1src = w1_in.ap().rearrange("(t p) f -> p t f", p=128)
        w2src = w2_in.ap().rearrange("(t p) f -> p t f", p=128)
        rs_dst = rs_in.ap().rearrange("c (t p) s -> p c t s", p=128)

        for rep in range(reps):
            R = f"r{rep}"
            # long-lived pools, opened in reverse order of their close time
            # (pool releases must be LIFO)
            w1_es = ExitStack()
            w1p = w1_es.enter_context(tc.tile_pool(name=f"w1p{R}", bufs=2))
            h2b_es = ExitStack()
            h2b_pool = h2b_es.enter_context(tc.tile_pool(name=f"h2bp{R}",
                                                         bufs=1))
            # ============ Phase 1: load x, transpose, LN1 -> hTb ==============
            hTb_es = ExitStack()
            hTb_pool = hTb_es.enter_context(tc.tile_pool(name=f"hbp{R}",
                                                         bufs=1))
            hTb = hTb_pool.tile([128, NDT, 512], bf16, name="hTb")
            with ExitStack() as ph1:
                big1 = ph1.enter_context(tc.tile_pool(name=f"big1{R}",
                                                      bufs=1))
                xrows = ph1.enter_context(tc.tile_pool(name=f"xrows{R}",
                                                       bufs=2))
                ps_s = ph1.enter_context(tc.tile_pool(name=f"ps_s{R}",
                                                      bufs=1, space="PSUM"))
                lnw = ph1.enter_context(tc.tile_pool(name=f"lnw{R}", bufs=3))
                rowp = ph1.enter_context(tc.tile_pool(name=f"row1{R}",
                                                      bufs=1))
                ln1g_r, ln1b_r = load_rows(rowp, f"ln1r{R}", ln1g_in,
                                           ln1b_in)
                s1 = ps_s.tile([1, 512], f32, name="ln1_s1")
                s2 = ps_s.tile([1, 512], f32, name="ln1_s2")
                xT = big1.tile([128, NDT, 512], f32r, name="xT")
                with ExitStack() as tps:
                    ps_tp = tps.enter_context(tc.tile_pool(
                        name=f"ps_tp{R}", bufs=4, space="PSUM"))
                    for st in range(4):
                        xrow = xrows.tile([128, D], f32, name="xrow")
                        for qu in range(4):
                            nc.sync.dma_start(
                                out=xrow[:, qu * 512:(qu + 1) * 512],
                                in_=x_in[st * 128:(st + 1) * 128,
                                         qu * 512:(qu + 1) * 512])
                        for t in range(NDT):
                            tp = ps_tp.tile([128, 128], f32, name="tp")
                            nc.tensor.transpose(
                                tp[:], xrow[:, t * 128:(t + 1) * 128],
                                ident32[:])
                            xcol = xT[:, t, st * 128:(st + 1) * 128]
                            nc.vector.tensor_copy(out=xcol, in_=tp[:])
                            _ln_stats(nc, f"ln1{R}", cst, lnw, s1, s2, xcol,
                                      t, t == 0, t == NDT - 1,
                                      cols=slice(st * 128, (st + 1) * 128))
                _ln_normalize(nc, tc, ph1, f"ln1{R}", cst,
                              lambda t: xT[:, t, :], hTb, ln1g_r, ln1b_r, s1,
                              s2)

            # ============ Phase 2: local QKV (all heads, own rows) ============
            # 12 groups of 4 ftiles; k = groups 0-3, q = 4-7, v = 8-11
            with ExitStack() as ph2:
                wq = ph2.enter_context(tc.tile_pool(name=f"wq{R}", bufs=3))
                kqv = ph2.enter_context(tc.tile_pool(name=f"kqv{R}", bufs=1))
                ps_qkv = ph2.enter_context(tc.tile_pool(name=f"ps_qkv{R}",
                                                        bufs=2, space="PSUM"))
                locs = [kqv.tile([128, NDT, 512], bf16, name=nm)
                        for nm in ("k_loc", "q_loc", "v_loc")]
                for g in range(12):
                    wch = wq.tile([128, NDT, 512], bf16, name="wch")
                    nc.sync.dma_start(
                        out=wch[:],
                        in_=wqkv_src[:, :, g * 512:(g + 1) * 512])
                    accs = [ps_qkv.tile([128, 512], f32, name=f"qa{ff}")
                            for ff in range(4)]
                    for t in range(NDT):
                        for ff in range(4):
                            nc.tensor.matmul(
                                accs[ff][:],
                                wch[:, t, ff * 128:(ff + 1) * 128],
                                hTb[:, t, :],
                                start=(t == 0), stop=(t == NDT - 1))
                    dst = locs[g // 4]
                    for ff in range(4):
                        ftg = g * 4 + ff
                        nc.scalar.activation(
                            out=dst[:, ftg % 16, :], in_=accs[ff][:],
                            func=AF.Identity,
                            bias=bqkv_c[:, ftg:ftg + 1], scale=1.0)
                    if g % 4 == 3:
                        which = g // 4
                        buf_in = (a2ak_in, a2aq_in, a2av_in)[which]
                        buf_out = (a2ak_out, a2aq_out, a2av_out)[which]
                        nc.sync.dma_start(
                            out=buf_in.ap().rearrange(
                                "c (u p) s -> p (c u) s", p=128),
                            in_=locs[which][:])
                        cc("AllToAll", ALU.bypass, ins=[buf_in.ap()],
                           outs=[buf_out.ap()])

            if upto < 2:
                hTb_es.close()
                h2b_es.close()
                w1_es.close()
                _early_out(nc, tc, R, y_out)
                continue

            # ============ Phase 3: attention + TP O-projection ================
            att_es = ExitStack()
            attp = att_es.enter_context(tc.tile_pool(name=f"attp{R}",
                                                     bufs=1))
            kT = attp.tile([128, HL, NB, 512], bf16, name="kT")
            qT = attp.tile([128, HL, NB, 512], bf16, name="qT")
            vT = attp.tile([128, HL, NB, 512], bf16, name="vT")
            ksrc = (a2ak_in if nocc else a2ak_out).ap().rearrange(
                "c (hh p) s -> p hh c s", p=128)
            qsrc = (a2aq_in if nocc else a2aq_out).ap().rearrange(
                "c (hh p) s -> p hh c s", p=128)
            vsrc = (a2av_in if nocc else a2av_out).ap().rearrange(
                "c (hh p) s -> p hh c s", p=128)
            for hh in range(HL):
                nc.sync.dma_start(out=kT[:, hh, :, :], in_=ksrc[:, hh, :, :])
            for hh in range(HL):
                nc.sync.dma_start(out=qT[:, hh, :, :], in_=qsrc[:, hh, :, :])
            for hh in range(HL):
                nc.sync.dma_start(out=vT[:, hh, :, :], in_=vsrc[:, hh, :, :])
            # Wo slice + first MLP W1 chunks: transfer during attention
            wos_es = ExitStack()
            wosp = wos_es.enter_context(tc.tile_pool(name=f"wosp{R}",
                                                     bufs=1))
            wos = wosp.tile([128, HL, D], bf16, name="wos")
            nc.sync.dma_start(out=wos[:],
                              in_=wo_in.ap().rearrange("h p f -> p h f"))
            w1pre = []
            for g in range(2):
                wt = w1p.tile([128, NDT, 512], bf16, name="w1c")
                nc.sync.dma_start(out=wt[:],
                                  in_=w1src[:, :, g * 512:(g + 1) * 512])
                w1pre.append(wt)

            if upto < 3:
                wos_es.close()
                att_es.close()
                hTb_es.close()
                h2b_es.close()
                w1_es.close()
                _early_out(nc, tc, R, y_out)
                continue

            with ExitStack() as ph4:
                aw = ph4.enter_context(tc.tile_pool(name=f"aw{R}",
                                                    bufs=RUNWAY + 3))
                fin = ph4.enter_context(tc.tile_pool(name=f"fin{R}", bufs=2))
                rst = ph4.enter_context(tc.tile_pool(name=f"rst{R}", bufs=2))
                ps_lg = ph4.enter_context(tc.tile_pool(name=f"ps_lg{R}",
                                                       bufs=1, space="PSUM"))
                ps_oa = ph4.enter_context(tc.tile_pool(name=f"ps_oa{R}",
                                                       bufs=2, space="PSUM"))
                ps_sm = ph4.enter_context(tc.tile_pool(name=f"ps_sm{R}",
                                                       bufs=1, space="PSUM"))
                ps_rb = ph4.enter_context(tc.tile_pool(name=f"ps_rb{R}",
                                                       bufs=1, space="PSUM"))
                ps_op = ph4.enter_context(tc.tile_pool(name=f"ps_op{R}",
                                                       bufs=2, space="PSUM"))
                oT = attp.tile([128, HL, NB, 512], bf16, name="oT")
                steps = []
                for b in range(NB):
                    seq, qb = b // QT, b % QT
                    for hh in range(HL):
                        nkt = 4 * (qb + 1)
                        for kt in range(nkt):
                            steps.append((b, seq, qb, hh, kt, nkt))

                p_tiles = {}
                accs = {}

                def emit_qk(i):
                    b, seq, qb, hh, kt, nkt = steps[i]
                    qcol = qT[:, hh, b, :]
                    kcol = kT[:, hh, seq * QT + kt // 4,
                              (kt % 4) * 128:(kt % 4) * 128 + 128]
                    lg = ps_lg.tile([128, 512], f32, name="lg")
                    nc.tensor.matmul(lg[:], kcol, qcol, start=True, stop=True)
                    p = aw.tile([128, 512], bf16, name="p")
                    nc.scalar.activation(out=p[:], in_=lg[:], func=AF.Exp,
                                         scale=INV_SQRT_HD)
                    j = kt - 4 * qb
                    if j >= 0:
                        nc.vector.tensor_mul(out=p[:], in0=p[:],
                                             in1=masks[j][:])
                    p_tiles[i] = p

                def emit_av(i):
                    b, seq, qb, hh, kt, nkt = steps[i]
                    if kt == 0:
                        accs[(b, hh)] = (
                            ps_oa.tile([128, 512], f32, name="oacc"),
                            ps_sm.tile([1, 512], f32, name="sacc"))
                    oacc, sacc = accs[(b, hh)]
                    p = p_tiles.pop(i)
                    vrow = vT[:, hh, seq * QT + kt // 4,
                              (kt % 4) * 128:(kt % 4) * 128 + 128]
                    nc.tensor.matmul(oacc[:], vrow, p[:],
                                     start=(kt == 0), stop=(kt == nkt - 1))
                    nc.tensor.matmul(sacc[:], ones128b[:], p[:],
                                     start=(kt == 0), stop=(kt == nkt - 1))
                    if kt == nkt - 1:
                        recip32 = fin.tile([1, 512], f32, name="recip32")
                        nc.vector.reciprocal(out=recip32[:], in_=sacc[:])
                        recip = fin.tile([1, 512], f32r, name="recip")
                        nc.scalar.activation(out=recip[:], in_=recip32[:],
                                             func=AF.Copy, scale=1.0)
                        rb = ps_rb.tile([128, 512], f32, name="rb")
                        nc.tensor.matmul(rb[:], cst["ones1x128r"][:],
                                         recip[:], start=True, stop=True)
                        ocol = oT[:, hh, b, :]
                        nc.scalar.activation(out=ocol, in_=oacc[:],
                                             func=AF.Copy, scale=1.0)
                        nc.vector.tensor_mul(out=ocol, in0=ocol,
                                             in1=rb[:])

                def emit_oproj(b):
                    # partial hp for rows of block b over my 256
                    # o-features; accumulate over my 2 heads
                    for dgrp in range(4):
                        stg = rst.tile([128, 4, 512], bf16, name="rstg")
                        for dd in range(4):
                            dt = dgrp * 4 + dd
                            op = ps_op.tile([128, 512], f32, name="op")
                            for hh in range(HL):
                                nc.tensor.matmul(
                                    op[:],
                                    wos[:, hh, dt * 128:(dt + 1) * 128],
                                    oT[:, hh, b, :],
                                    start=(hh == 0), stop=(hh == HL - 1))
                            nc.vector.tensor_copy(out=stg[:, dd, :],
                                                  in_=op[:])
                        nc.sync.dma_start(
                            out=rs_dst[:, b, dgrp * 4:dgrp * 4 + 4, :],
                            in_=stg[:])

                nq = min(RUNWAY, len(steps))
                for jj in range(nq):
                    emit_qk(jj)
                # in-place transpose of vT -> v, after the QK runway so the
                # in-order PE queue is not blocked on A2A-v before it.
                # 4 blocks per PSUM tile -> 1 bank, 1 wide copy per group.
                with ExitStack() as tp3s:
                    ps_tp2 = tp3s.enter_context(tc.tile_pool(
                        name=f"ps_tp2{R}", bufs=1, space="PSUM"))
                    for hh in range(HL):
                        for qb_ in range(NB):
                            tp = ps_tp2.tile([128, 512], bf16, name="tpv")
                            for sub in range(4):
                                off = sub * 128
                                nc.tensor.transpose(
                                    tp[:, off:off + 128],
                                    vT[:, hh, qb_, off:off + 128],
                                    identb[:])
                            nc.vector.tensor_copy(out=vT[:, hh, qb_, :],
                                                  in_=tp[:])
                for i in range(len(steps)):
                    if i + RUNWAY < len(steps):
                        emit_qk(i + RUNWAY)
                    emit_av(i)
                    b, seq, qb, hh, kt, nkt = steps[i]
                    if hh == HL - 1 and kt == nkt - 1:
                        emit_oproj(b)

            cc("ReduceScatter", ALU.add, ins=[rs_in.ap()], outs=[rs_out.ap()])
            wos_es.close()
            att_es.close()

            if upto < 4:
                hTb_es.close()
                h2b_es.close()
                w1_es.close()
                _early_out(nc, tc, R, y_out)
                continue

            # ============ Phase 4: residual + LN2 =============================
            h2Tb = h2b_pool.tile([128, NDT, 512], bf16, name="h2Tb")
            with ExitStack() as ph5:
                ow = ph5.enter_context(tc.tile_pool(name=f"ow{R}", bufs=1))
                rsb = ow.tile([128, NDT, 512], bf16, name="rsb")
                nc.sync.dma_start(
                    out=rsb[:],
                    in_=(rs_in.ap().rearrange(
                        "c (t p) s -> p c t s", p=128)[:, 0] if nocc
                        else rs_out.ap().rearrange("(t p) s -> p t s",
                                                   p=128)))
                rowp2 = ph5.enter_context(tc.tile_pool(name=f"row2{R}",
                                                       bufs=1))
                ln2g_r, ln2b_r = load_rows(rowp2, f"ln2r{R}", ln2g_in,
                                           ln2b_in)
                hpost = ow.tile([128, NDT, 512], f32r, name="hpost")
                for dt in range(NDT):
                    # hpost = (rsb + bo) + hTb
                    nc.vector.scalar_tensor_tensor(
                        out=hpost[:, dt, :], in0=rsb[:, dt, :],
                        scalar=bo_c[:, dt:dt + 1],
                        in1=hTb[:, dt, :],
                        op0=ALU.add, op1=ALU.add)
                _ln_feature_major(nc, tc, ph5, f"ln2{R}", cst,
                                  lambda t: hpost[:, t, :], h2Tb,
                                  ln2g_r, ln2b_r)
            hTb_es.close()

            if upto < 5:
                h2b_es.close()
                w1_es.close()
                _early_out(nc, tc, R, y_out)
                continue

            # ============ Phase 5a: MLP up-proj + gelu ========================
            g_es = ExitStack()
            g_pool = g_es.enter_context(tc.tile_pool(name=f"gp{R}", bufs=1))
            gt = g_pool.tile([128, NF1, 512], bf16, name="gt")
            with ExitStack() as ph6:
                ps_m = ph6.enter_context(tc.tile_pool(name=f"ps_m{R}",
                                                      bufs=2, space="PSUM"))
                for g in range(16):
                    if g < 2:
                        wch = w1pre[g]
                    else:
                        wch = w1p.tile([128, NDT, 512], bf16, name="w1c")
                        nc.sync.dma_start(
                            out=wch[:],
                            in_=w1src[:, :, g * 512:(g + 1) * 512])
                    accs = [ps_m.tile([128, 512], f32, name=f"ma{ff}")
                            for ff in range(4)]
                    for t in range(NDT):
                        for ff in range(4):
                            nc.tensor.matmul(
                                accs[ff][:],
                                wch[:, t, ff * 128:(ff + 1) * 128],
                                h2Tb[:, t, :],
                                start=(t == 0), stop=(t == NDT - 1))
                    for ff in range(4):
                        f1 = g * 4 + ff
                        nc.scalar.activation(
                            out=gt[:, f1, :], in_=accs[ff][:],
                            func=AF.Gelu_apprx_tanh,
                            bias=b1_c[:, f1:f1 + 1], scale=1.0)
            if upto < 6:
                g_es.close()
                h2b_es.close()
                w1_es.close()
                _early_out(nc, tc, R, y_out)
                continue

            # ============ Phase 5b: MLP down-proj + residual + store ==========
            with ExitStack() as ph7:
                w2p = ph7.enter_context(tc.tile_pool(name=f"w2p{R}", bufs=2))
                ost = ph7.enter_context(tc.tile_pool(name=f"ost{R}", bufs=2))
                ys = ph7.enter_context(tc.tile_pool(name=f"ys{R}", bufs=2))
                ps_m2 = ph7.enter_context(tc.tile_pool(name=f"ps_m2{R}",
                                                       bufs=1, space="PSUM"))
                ps_tp3 = ph7.enter_context(tc.tile_pool(name=f"ps_tp3{R}",
                                                        bufs=2, space="PSUM"))
                for dg in range(4):
                    accs = [ps_m2.tile([128, 512], f32, name=f"mb{dd}")
                            for dd in range(4)]
                    for quar in range(4):
                        wch = w2p.tile([128, 16, 512], bf16, name="w2c")
                        nc.sync.dma_start(
                            out=wch[:],
                            in_=w2src[:, quar * 16:(quar + 1) * 16,
                                      dg * 512:(dg + 1) * 512])
                        for fi in range(16):
                            ft = quar * 16 + fi
                            for dd in range(4):
                                nc.tensor.matmul(
                                    accs[dd][:],
                                    wch[:, fi, dd * 128:(dd + 1) * 128],
                                    gt[:, ft, :],
                                    start=(ft == 0), stop=(ft == NF1 - 1))
                    outg = ost.tile([128, 4, 512], f32, name="outg")
                    for dd in range(4):
                        dt = dg * 4 + dd
                        # outg = (m2 + b2) + h2Tb
                        nc.vector.scalar_tensor_tensor(
                            out=outg[:, dd, :], in0=accs[dd][:],
                            scalar=b2_c[:, dt:dt + 1],
                            in1=h2Tb[:, dt, :],
                            op0=ALU.add, op1=ALU.add)
                    ystage = ys.tile([128, 4, 512], f32, name="ystage")
                    for dd in range(4):
                        for rseg in range(4):
                            tp = ps_tp3.tile([128, 128], f32, name="tpo")
                            nc.tensor.transpose(
                                tp[:],
                                outg[:, dd, rseg * 128:(rseg + 1) * 128],
                                ident32[:])
                            nc.vector.tensor_copy(
                                out=ystage[:, rseg, dd * 128:(dd + 1) * 128],
                                in_=tp[:])
                    for rseg in range(4):
                        nc.sync.dma_start(
                            out=y_out[rseg * 128:(rseg + 1) * 128,
                                      dg * 512:(dg + 1) * 512],
                            in_=ystage[:, rseg, :])
            g_es.close()
            h2b_es.close()
            w1_es.close()

    _split_multiwaits(nc)
    return nc


def _split_multiwaits(nc, max_waits=1):
    """walrus in this toolchain rejects >1 sem-wait on most instruction
    structs; split extras onto preceding sequencer NoOps (same engine)."""
    if getattr(nc, "_skip_split_multiwaits", False):
        return
    for fn in nc.m.functions:
        for bb in fn.blocks:
            new_list, changed = [], False
            for inst in bb.instructions:
                si = inst.sync_info
                lim = max_waits
                if si is not None and len(si.on_wait) > lim:
                    waits = list(si.on_wait)
                    for k, w in enumerate(waits[:-lim]):
                        nop = mybir.InstNoOp(name=f"{inst.name}-splitw{k}")
                        nop.engine = inst.engine
                        nop.sync_info = mybir.SyncInfo(on_wait=[w],
                                                       on_update=[])
                        new_list.append(nop)
                    inst.sync_info = mybir.SyncInfo(
                        on_wait=waits[-lim:],
                        on_update=list(si.on_update))
                    changed = True
                new_list.append(inst)
            if changed:
                bb.instructions = new_list


# ---------------------------------------------------------------------------
# Persistent SPMD runner (compile once per process, reuse executable).

class SpmdKernel:
    def __init__(self, nc, n_cores):
        import jax
        from jax.sharding import Mesh, PartitionSpec
        from jax.experimental.shard_map import shard_map
        from concourse.bass2jax import (_bass_exec_p, install_neuronx_cc_hook,
                                        partition_id_tensor)
        self.jax = jax
        self.PartitionSpec = PartitionSpec
        install_neuronx_cc_hook()
        self.nc = nc
        self.n_cores = n_cores
        partition_name = (nc.partition_id_tensor.name
                          if nc.partition_id_tensor else None)
        in_names, out_names, out_avals, zero_outs = [], [], [], []
        for alloc in nc.m.functions[0].allocations:
            if not isinstance(alloc, mybir.MemoryLocationSet):
                continue
            name = alloc.memorylocations[0].name
            if alloc.kind == "ExternalInput":
                if name != partition_name:
                    in_names.append(name)
            elif alloc.kind == "ExternalOutput":
                shape = tuple(alloc.tensor_shape)
                dtype = mybir.dt.np(alloc.dtype)
                out_names.append(name)
                out_avals.append(jax.core.ShapedArray(shape, dtype))
                zero_outs.append(np.zeros(shape, dtype))
        n_params = len(in_names)
        n_outs = len(out_avals)
        all_in_names = list(in_names) + list(out_names)
        if partition_name is not None:
            all_in_names.append(partition_name)
        self.in_names = in_names
        self.out_names = out_names
        self.out_avals = out_avals
        self.zero_outs = zero_outs
        self.n_params = n_params

        def _body(*args):
            operands = list(args)
            if partition_name is not None:
                operands.append(partition_id_tensor())
            outs = _bass_exec_p.bind(
                *operands,
                out_avals=tuple(out_avals),
                in_names=tuple(all_in_names),
                out_names=tuple(out_names),
                lowering_input_output_aliases=(),
                sim_require_finite=True,
                sim_require_nnan=True,
                nc=nc,
            )
            return tuple(outs)

        devices = jax.devices()[:n_cores]
        assert len(devices) == n_cores
        self.mesh = Mesh(np.asarray(devices), ("core",))
        in_specs = (PartitionSpec("core"),) * (n_params + n_outs)
        out_specs = (PartitionSpec("core"),) * n_outs
        self.fn = jax.jit(
            shard_map(_body, mesh=self.mesh, in_specs=in_specs,
                      out_specs=out_specs, check_rep=False),
            keep_unused=True,
        )

    def stage_inputs(self, in_maps):
        from jax.sharding import NamedSharding
        per_core = [[np.asarray(m[name]) for name in self.in_names]
                    for m in in_maps]
        concat_in = [
            np.ascontiguousarray(np.concatenate(
                [per_core[c][i] for c in range(self.n_cores)], axis=0))
            for i in range(self.n_params)
        ]
        concat_zeros = [
            np.zeros((self.n_cores * z.shape[0], *z.shape[1:]), z.dtype)
            for z in self.zero_outs
        ]
        sh = NamedSharding(self.mesh, self.PartitionSpec("core"))
        return [self.jax.device_put(a, sh) for a in (concat_in + concat_zeros)]

    def run_staged(self, args):
        outs = self.fn(*args)
        self.jax.block_until_ready(outs)
        return outs

    def results(self, outs):
        res = []
        for c in range(self.n_cores):
            res.append({
                name: np.asarray(outs[i]).reshape(
                    self.n_cores, *self.out_avals[i].shape)[c]
                for i, name in enumerate(self.out_names)
            })
        return res

    def __call__(self, in_maps):
        return self.results(self.run_staged(self.stage_inputs(in_maps)))


_NC_CACHE = {}


def get_runner(reps=1, upto=6, nocc=False):
    key = f"runner{reps}-{upto}-{nocc}"
    if key not in _NC_CACHE:
        nc = build_nc(reps, upto, nocc)
        _NC_CACHE[key] = SpmdKernel(nc, N_CORES)
    return _NC_CACHE[key]


def host_prep(inputs):
    import ml_dtypes
    bf = ml_dtypes.bfloat16

    def a32(v):
        return np.asarray(v, np.float32)
    x = a32(inputs["x"])
    Wq, Wk, Wv = a32(inputs["Wq"]), a32(inputs["Wk"]), a32(inputs["Wv"])
    Wo = a32(inputs["Wo"])
    W1, W2 = a32(inputs["W1"]), a32(inputs["W2"])
    x_flat = np.ascontiguousarray(x.reshape(NROW, D))
    # columns [Wk.T | Wq.T | Wv.T], features head-major
    wqkvT = np.ascontiguousarray(
        np.concatenate([Wk.T, Wq.T, Wv.T], axis=1).astype(bf))
    bqkv = np.concatenate([a32(inputs["bk"]), a32(inputs["bq"]),
                           a32(inputs["bv"])])
    woT = Wo.T.astype(bf)            # [in-feature, dout]
    w1T = np.ascontiguousarray(W1.T.astype(bf))
    w2T = np.ascontiguousarray(W2.T.astype(bf))
    shared = {
        "wqkvT": wqkvT, "bqkv": bqkv, "w1T": w1T, "w2T": w2T,
        "bo": a32(inputs["bo"]), "b1": a32(inputs["b1"]),
        "b2": a32(inputs["b2"]),
        "ln1g": a32(inputs["ln1_g"]), "ln1b": a32(inputs["ln1_b"]),
        "ln2g": a32(inputs["ln2_g"]), "ln2b": a32(inputs["ln2_b"]),
    }
    in_maps = []
    for c in range(N_CORES):
        m = dict(shared)
        m["x"] = np.ascontiguousarray(x_flat[ROWS * c: ROWS * (c + 1)])
        # Wo.T rows for my o-features (heads 2c, 2c+1), tiled [2, 128, D]
        m["woS"] = np.ascontiguousarray(
            woT[256 * c:256 * (c + 1)].reshape(HL, 128, D))
        in_maps.append(m)
    return in_maps


def kernel(**inputs) -> np.ndarray:
    in_maps = host_prep(inputs)
    runner = get_runner()
    res = runner(in_maps)
    out = np.concatenate([res[c]["y"] for c in range(N_CORES)], axis=0)
    return out.reshape(B, S, D)

